# revision 1
# baseline (speedup 1.0000x reference)
"""Trainium2 Bass kernel for nn_Model2_8340826488964 (dense_mlp, recurrent+syncBN).

Model per timestep t (T=512, B=2048, NH=1024, NV=31):
    h = relu((h_prev + emb[x_t]) @ W_hh.T + b_hh)
    BN over batch (training stats), out_t = BN(h) @ W_ho.T + b_ho

Strategy: data-parallel over batch (256 rows/core on 8 cores).
 - Layout: features on partitions (8 f-tiles x 128), batch on free axis (256).
 - Recurrence in bf16 (error saturates ~6e-3, well under tolerance).
 - emb-add folded into the matmul via e2 = emb @ W_hh.T and a host-precomputed
   one-hot streamed from HBM: z = h@W_hh.T + onehot.T@e2 + b  (PE-only).
 - BN stats: ACT relu pass emits per-feature sums (accum_out); DVE
   tensor_tensor_reduce emits sum-of-squares. Stats for G steps are batched
   into ONE AllReduce (collective runs on TOPSP, fully overlapped).
 - mm2 for step t executes at step t+G+1 (after its stats arrive): DVE
   normalizes hn = a*h + c, PE computes out2[b,31] = hn.T @ W_ho.T slices in
   output orientation (batch on partitions), DVE adds b_ho and accumulates
   into an SBUF buffer DMA'd to HBM per group.
"""
import sys, os
sys.path.insert(0, "/opt/trn_rl_repo")
import numpy as np
import ml_dtypes

from concourse import bass, bacc, tile, bass_utils
from concourse import mybir
from concourse.bass_interp import get_hw_module

BF16 = ml_dtypes.bfloat16

N_CORES = 8
B, T_FULL, NH, NV = 2048, 512, 1024, 31
BC = B // N_CORES            # 256 batch rows per core
NF = NH // 128               # 8 feature tiles
BN_EPS = 1e-5

G = 8                        # steps per stats-allreduce group
D = 12                       # h ring depth (must be > G+2)

F32 = mybir.dt.float32
BF = mybir.dt.bfloat16
AF = mybir.ActivationFunctionType
OP = mybir.AluOpType


def build(T: int, g: int = G, d: int = D, no_cc: bool = False, no_sqrt: bool = False, level: int = 4):
    assert T % g == 0
    nc = bacc.Bacc("TRN2", target_bir_lowering=False, debug=False,
                   enable_asserts=False, num_devices=N_CORES)

    whh_d = nc.dram_tensor("whh", [128, 64 * 128], BF, kind="ExternalInput").ap()
    e2_d = nc.dram_tensor("e2t", [NV, NH], BF, kind="ExternalInput").ap()
    whot_d = nc.dram_tensor("whot", [128, NF * NV], BF, kind="ExternalInput").ap()
    bhh_d = nc.dram_tensor("bhh", [128, NF], F32, kind="ExternalInput").ap()
    gam_d = nc.dram_tensor("gam", [128, NF], F32, kind="ExternalInput").ap()
    bet_d = nc.dram_tensor("bet", [128, NF], F32, kind="ExternalInput").ap()
    bho_d = nc.dram_tensor("bho", [128, NV], F32, kind="ExternalInput").ap()
    oneh_d = nc.dram_tensor("oneh", [NV, T * BC], BF, kind="ExternalInput").ap()
    out_d = nc.dram_tensor("out_shard", [BC, T, NV], F32, kind="ExternalOutput").ap()

    n_groups = T // g
    inv_n = 1.0 / float(B)

    with tile.TileContext(nc) as tc:
        with tc.tile_pool(name="const", bufs=1) as cpool, \
             tc.tile_pool(name="hring", bufs=d) as hpool, \
             tc.tile_pool(name="hn", bufs=2) as hnpool, \
             tc.tile_pool(name="oneh", bufs=3) as opool, \
             tc.tile_pool(name="stats", bufs=2) as spool, \
             tc.tile_pool(name="fin", bufs=2) as fpool, \
             tc.tile_pool(name="ac", bufs=3) as acpool, \
             tc.tile_pool(name="acc", bufs=2) as accpool, \
             tc.tile_pool(name="scratch", bufs=1) as scpool, \
             tc.tile_pool(name="ps1", bufs=3, space="PSUM") as ps1pool, \
             tc.tile_pool(name="ps2", bufs=2, space="PSUM") as ps2pool, \
             tc.tile_pool(name="dram", bufs=4, space="DRAM") as dpool:

            # ---- load constants ----
            whh = cpool.tile([128, 64 * 128], BF, tag="whh", name="whh")
            e2 = cpool.tile([NV, NH], BF, tag="e2", name="e2")
            whot = cpool.tile([128, NF * NV], BF, tag="whot", name="whot")
            bhh = cpool.tile([128, NF], F32, tag="bhh", name="bhh")
            gam = cpool.tile([128, NF], F32, tag="gam", name="gam")
            bet = cpool.tile([128, NF], F32, tag="bet", name="bet")
            bho = cpool.tile([128, NV], F32, tag="bho", name="bho")
            nc.sync.dma_start(whh[:], whh_d[:])
            nc.sync.dma_start(e2[:], e2_d[:])
            nc.sync.dma_start(whot[:], whot_d[:])
            nc.sync.dma_start(bhh[:], bhh_d[:])
            nc.sync.dma_start(gam[:], gam_d[:])
            nc.sync.dma_start(bet[:], bet_d[:])
            nc.sync.dma_start(bho[:], bho_d[:])

            sq_scr = scpool.tile([128, 256], BF, tag="sqscr", name="sqscr")

            h_tiles = {}        # s -> h tile [128, NF*256] bf16
            oneh_tiles = {}     # group -> [NV, g*256] bf16
            stats_loc = {}      # group -> [128, 16*g] f32 (sums | sumsqs)
            stats_glb = {}      # group -> [128, 16*g] f32
            rsq_mean = {}       # group -> (rsq [128,8g], mean [128,8g])
            acc_tiles = {}      # group -> [128, 2*g*NV] f32
            cc_done = {}

            # prefetch onehot for groups 0,1
            for gg in range(min(2, n_groups)):
                ot = opool.tile([NV, g * BC], BF, tag="oneh", name="oneh")
                nc.sync.dma_start(ot[:], oneh_d[:, gg * g * BC:(gg + 1) * g * BC])
                oneh_tiles[gg] = ot

            for s in range(T + g + 1):
                # ======== forward recurrence step s ========
                if s < T:
                    u, gg = s % g, s // g
                    if u == 0:
                        stats_loc[gg] = spool.tile([128, 16 * g], F32, tag="sloc", name="sloc")
                        if gg + 2 < n_groups:
                            ot = opool.tile([NV, g * BC], BF, tag="oneh", name="oneh")
                            nc.sync.dma_start(
                                ot[:], oneh_d[:, (gg + 2) * g * BC:(gg + 3) * g * BC])
                            oneh_tiles[gg + 2] = ot
                    sloc = stats_loc[gg]
                    h_t = hpool.tile([128, NF * 256], BF, tag="h", name="h")
                    h_tiles[s] = h_t
                    h_prev = h_tiles.get(s - 1)
                    oneh_g = oneh_tiles[gg]

                    for half in range(2):
                        psh = ps1pool.tile([128, 1024], F32, tag="ps1", name="ps1")
                        for q in range(4):
                            fi = half * 4 + q
                            pslice = psh[:, q * 256:(q + 1) * 256]
                            if s > 0:
                                for ki in range(NF):
                                    nc.tensor.matmul(
                                        pslice,
                                        whh[:, (ki * NF + fi) * 128:(ki * NF + fi + 1) * 128],
                                        h_prev[:, ki * 256:(ki + 1) * 256],
                                        start=(ki == 0), stop=False)
                            nc.tensor.matmul(
                                pslice,
                                e2[:, fi * 128:(fi + 1) * 128],
                                oneh_g[:, u * BC:(u + 1) * BC],
                                start=(s == 0), stop=True)
                            # ACT: h = relu(psum + b), accumulate per-feature sum
                            if level >= 1:
                                nc.scalar.activation(
                                    h_t[:, fi * 256:(fi + 1) * 256], pslice,
                                    AF.Relu, bias=bhh[:, fi:fi + 1], scale=1.0,
                                    accum_out=sloc[:, u * 8 + fi:u * 8 + fi + 1])
                                nc.vector.scalar_tensor_tensor(
                                    sq_scr[:], h_t[:, fi * 256:(fi + 1) * 256], 1.0,
                                    h_t[:, fi * 256:(fi + 1) * 256],
                                    op0=OP.mult, op1=OP.mult,
                                    accum_out=sloc[:, 8 * g + u * 8 + fi:8 * g + u * 8 + fi + 1])
                            else:
                                nc.scalar.activation(
                                    h_t[:, fi * 256:(fi + 1) * 256], pslice,
                                    AF.Relu, bias=bhh[:, fi:fi + 1], scale=1.0)

                    if u == g - 1 and level >= 3:
                        # group complete: allreduce the stats
                        cin = dpool.tile([128, 16 * g], F32, tag="ccin", name="ccin")
                        cout = dpool.tile([128, 16 * g], F32, tag="ccout", name="ccout",
                                          addr_space="Shared")
                        nc.gpsimd.dma_start(cin[:], sloc[:])
                        if no_cc:
                            nc.gpsimd.dma_start(cout[:], cin[:])
                        else:
                            nc.gpsimd.collective_compute(
                                "AllReduce", OP.add, ins=[cin[:]], outs=[cout[:]],
                                replica_groups=[list(range(N_CORES))])
                        sg = spool.tile([128, 16 * g], F32, tag="sglb", name="sglb")
                        nc.gpsimd.dma_start(sg[:], cout[:])
                        stats_glb[gg] = sg
                    elif u == g - 1:
                        stats_glb[gg] = sloc

                # ======== delayed BN + output path for step t = s-g-1 ========
                t = s - g - 1
                if level < 2:
                    if s == T + g and level >= 0:
                        nc.gpsimd.dma_start(out_d[0:128, 0:1, :],
                                            h_tiles[T - 1][:, 0:31])
                    continue_delayed = False
                else:
                    continue_delayed = True
                if continue_delayed and 0 <= t < T:
                    ut, gt = t % g, t // g
                    if ut == 0 and level == 2:
                        rsq = fpool.tile([128, 8 * g], F32, tag="rsq", name="rsq")
                        mean = fpool.tile([128, 8 * g], F32, tag="mean", name="mean")
                        nc.vector.memset(rsq[:], 1.0)
                        nc.vector.memset(mean[:], 0.0)
                        rsq_mean[gt] = (rsq, mean)
                        acc_tiles[gt] = accpool.tile([128, 2 * g * NV], F32, tag="acc", name="acc")
                    if ut == 0 and level >= 3:
                        # finalize group stats: mean, var, rsqrt(var+eps)
                        sg = stats_glb[gt]
                        mean = fpool.tile([128, 8 * g], F32, tag="mean", name="mean")
                        ex2 = fpool.tile([128, 8 * g], F32, tag="ex2", name="ex2")
                        vep = fpool.tile([128, 8 * g], F32, tag="vep", name="vep")
                        rcp = fpool.tile([128, 8 * g], F32, tag="rcp", name="rcp")
                        rsq = fpool.tile([128, 8 * g], F32, tag="rsq", name="rsq")
                        nc.vector.tensor_scalar(mean[:], sg[:, 0:8 * g], inv_n, None, OP.mult)
                        nc.vector.tensor_scalar(ex2[:], sg[:, 8 * g:16 * g], inv_n, None, OP.mult)
                        # vep = ex2 - mean^2 + eps ; rsq = sqrt(1/vep)
                        m2 = fpool.tile([128, 8 * g], F32, tag="m2", name="m2")
                        nc.vector.tensor_tensor(m2[:], mean[:], mean[:], OP.mult)
                        nc.vector.scalar_tensor_tensor(
                            vep[:], ex2[:], BN_EPS, m2[:], op0=OP.add, op1=OP.subtract)
                        nc.vector.reciprocal(rcp[:], vep[:])
                        if no_sqrt:
                            nc.vector.tensor_copy(rsq[:], rcp[:])
                        else:
                            nc.scalar.activation(rsq[:], rcp[:], AF.Sqrt)
                        rsq_mean[gt] = (rsq, mean)
                        acc_tiles[gt] = accpool.tile([128, 2 * g * NV], F32, tag="acc", name="acc")

                    rsq, mean = rsq_mean[gt]
                    a_u = acpool.tile([128, 8], F32, tag="a_u", name="a_u")
                    c_u = acpool.tile([128, 8], F32, tag="c_u", name="c_u")
                    nc.vector.tensor_tensor(a_u[:], rsq[:, ut * 8:(ut + 1) * 8], gam[:], OP.mult)
                    nc.vector.tensor_tensor(c_u[:], mean[:, ut * 8:(ut + 1) * 8], a_u[:], OP.mult)
                    nc.vector.tensor_tensor(c_u[:], bet[:], c_u[:], OP.subtract)

                    h_old = h_tiles.pop(t)
                    hn = hnpool.tile([128, NF * 256], BF, tag="hn", name="hn")
                    for fi in range(NF):
                        nc.vector.tensor_scalar(
                            hn[:, fi * 256:(fi + 1) * 256],
                            h_old[:, fi * 256:(fi + 1) * 256],
                            a_u[:, fi:fi + 1], c_u[:, fi:fi + 1],
                            op0=OP.mult, op1=OP.add)

                    ps2 = ps2pool.tile([128, 2 * NV], F32, tag="ps2", name="ps2")
                    for j in range(2):
                        for fi in range(NF):
                            nc.tensor.matmul(
                                ps2[:, j * NV:(j + 1) * NV],
                                hn[:, fi * 256 + j * 128:fi * 256 + (j + 1) * 128],
                                whot[:, fi * NV:(fi + 1) * NV],
                                start=(fi == 0), stop=(fi == NF - 1))
                    acc = acc_tiles[gt]
                    for j in range(2):
                        nc.vector.tensor_tensor(
                            acc[:, (j * g + ut) * NV:(j * g + ut + 1) * NV],
                            ps2[:, j * NV:(j + 1) * NV], bho[:, 0:NV], OP.add)

                    if ut == g - 1:
                        for j in range(2):
                            nc.sync.dma_start(
                                out_d[j * 128:(j + 1) * 128, gt * g:(gt + 1) * g, :],
                                acc[:, j * g * NV:(j + 1) * g * NV])
                        del acc_tiles[gt], stats_glb[gt], rsq_mean[gt], stats_loc[gt]
                        if gt in oneh_tiles:
                            del oneh_tiles[gt]

    nc.compile()
    nc.m = get_hw_module(nc.m)
    return nc


def prep_inputs(x, emb, W_hh, b_hh, W_ho, b_ho, gamma, beta, T):
    """Host-side packing. Returns in_maps (list of per-core dicts)."""
    x = np.asarray(x)
    emb = np.asarray(emb, np.float32)
    W_hh = np.asarray(W_hh, np.float32)
    b_hh = np.asarray(b_hh, np.float32)
    W_ho = np.asarray(W_ho, np.float32)
    b_ho = np.asarray(b_ho, np.float32)
    gamma = np.asarray(gamma, np.float32)
    beta = np.asarray(beta, np.float32)

    WT = np.ascontiguousarray(W_hh.T)                      # [k, f]
    whh = WT.reshape(NF, 128, NF, 128).transpose(1, 0, 2, 3)  # [k_l, ki, fi, f_l]
    whh = np.ascontiguousarray(whh.reshape(128, 64 * 128)).astype(BF16)
    e2 = (emb @ W_hh.T).astype(BF16)                       # [31, 1024]
    whot = np.ascontiguousarray(
        W_ho.T.reshape(NF, 128, NV).transpose(1, 0, 2).reshape(128, NF * NV)).astype(BF16)
    bhh = np.ascontiguousarray(b_hh.reshape(NF, 128).T)    # [128, 8]
    gam = np.ascontiguousarray(gamma.reshape(NF, 128).T)
    bet = np.ascontiguousarray(beta.reshape(NF, 128).T)
    bho = np.broadcast_to(b_ho[None, :], (128, NV)).copy() # [128, 31]

    common = dict(whh=whh, e2t=e2, whot=whot, bhh=bhh, gam=gam, bet=bet, bho=bho)
    in_maps = []
    t_idx = np.arange(T)[:, None]
    b_idx = np.arange(BC)[None, :]
    for c in range(N_CORES):
        xc = x[c * BC:(c + 1) * BC, :T].T                  # [T, 256]
        oh = np.zeros((NV, T, BC), dtype=BF16)
        oh[xc, t_idx, b_idx] = 1
        m = dict(common)
        m["oneh"] = oh.reshape(NV, T * BC)
        in_maps.append(m)
    return in_maps


_CACHE = {}

def _get_built(T):
    if T not in _CACHE:
        _CACHE[T] = build(T)
    return _CACHE[T]


def run(inputs, T=T_FULL, trace=False):
    nc = _get_built(T)
    in_maps = prep_inputs(inputs["x"], inputs["emb"], inputs["W_hh"], inputs["b_hh"],
                          inputs["W_ho"], inputs["b_ho"], inputs["gamma"],
                          inputs["beta"], T)
    res = bass_utils.run_bass_kernel_spmd(
        nc, in_maps, core_ids=list(range(N_CORES)), trace=trace)
    out = np.concatenate([res.results[c]["out_shard"] for c in range(N_CORES)], axis=0)
    return out, res


def kernel(**inputs) -> np.ndarray:
    out, _ = run(inputs, T=T_FULL, trace=False)
    return out

